# revision 10
# baseline (speedup 1.0000x reference)
"""Trainium2 Bass kernel for an AttentionBlock (GroupNorm + 1x1-conv QKV +
4-head attention over 48x48 pixels + 1x1-conv proj + residual).

Contract: kernel(**inputs) takes the FULL unsharded inputs (as produced by
setup_inputs) and returns the FULL output (8, 256, 48, 48) float32.

Strategy: data-parallel over batch — batch element i runs on NeuronCore i.
All parameters are replicated. Per core everything stays on-chip:

  x [256, 2304] (channels on partitions)
  -> GroupNorm via per-channel bn_stats + group-averaging matmul -> affine A,B
  -> q,k = W_qk @ xn  (channels-on-partitions layout; q pre-scaled by 1/8)
  -> vT  = xn^T @ W_v (pixels-on-partitions layout, so PV needs no transpose),
     augmented with a ones column per head (computes softmax denominators for
     free inside the PV matmul)
  -> scores S^T[j,i] = k_j . q_i  (j pixel-block on PSUM partitions),
     exp on ScalarE (no max subtraction; |S|<=~30 so exp is fp32-safe),
     E stored bf16
  -> PV[d,i] = sum_j vT[j,d] E[j,i] accumulated over 18 j-blocks in PSUM;
     softmax division deferred: out rows are unnormalized, denominators ride
     along in a spare PSUM row
  -> normalize by broadcasting 1/sums over the 64 rows of each head via a
     tiny selection matmul
  -> proj + residual + (v-bias folded into an effective proj bias on host)

Matmuls run as float32r (full PE rate); E/vT are bf16.
"""

import math
from contextlib import ExitStack

import numpy as np

import concourse.bacc as bacc
import concourse.bass as bass
import concourse.mybir as mybir
import concourse.tile as tile
from concourse.bass_utils import run_bass_kernel_spmd

F32 = mybir.dt.float32
F32R = mybir.dt.float32r
BF16 = mybir.dt.bfloat16
AF = mybir.ActivationFunctionType
OP = mybir.AluOpType

N_CORES = 8
C = 256          # channels
HW = 2304        # 48*48 pixels
NH = 4           # heads
HD = 64          # head dim
G = 32           # groupnorm groups
EPS = 1e-5
CT = 2           # channel partition tiles of 128
PB = 18          # pixel blocks of 128

# pixel chunks for N<=512 matmuls
PCH = [(0, 512), (512, 512), (1024, 512), (1536, 512), (2048, 256)]
# i-chunks for the attention stage (PSUM-bank friendly: 2+2+1 banks)
ICH = [(0, 1024), (1024, 1024), (2048, 256)]

# ones-column position (within each head's 128-col lhsT block) = the PSUM
# partition the softmax denominator lands on. Engine APs must start at a
# partition in {0,32,64,96}; even heads' data sits at partitions 0-63 so their
# denominators go to 64/96, odd heads' data sits at 64-127 so theirs go to 0/32.
ONES_COL = {0: 64, 1: 0, 2: 96, 3: 32}


def _chunks(length):
    out = []
    off = 0
    while off < length:
        cl = min(512, length - off)
        out.append((off, cl))
        off += cl
    return out


def _build():
    nc = bacc.Bacc(
        "TRN2", target_bir_lowering=False, debug=False, num_devices=N_CORES
    )
    x_d = nc.dram_tensor("x", [C, HW], F32, kind="ExternalInput")
    wqkvT_d = nc.dram_tensor("wqkvT", [C, 3 * C], F32R, kind="ExternalInput")
    wprojT_d = nc.dram_tensor("wprojT", [C, C], F32R, kind="ExternalInput")
    gsel_d = nc.dram_tensor("gsel", [C, C], F32, kind="ExternalInput")
    bsel_d = nc.dram_tensor("bsel", [NH, C], F32R, kind="ExternalInput")
    # per-channel vectors: [...,0]=gn_w [...,1]=gn_b [...,2]=qb/8 [...,3]=kb
    # [...,4]=proj_b + proj_w @ v_bias
    vecs_d = nc.dram_tensor("vecs", [CT, 128, 5], F32, kind="ExternalInput")
    vmask_d = nc.dram_tensor("vmask", [128, 4 * 128], BF16, kind="ExternalInput")
    out_d = nc.dram_tensor("out", [C, HW], F32, kind="ExternalOutput")

    with ExitStack() as ctx:
        tc = ctx.enter_context(tile.TileContext(nc))
        const = ctx.enter_context(tc.tile_pool(name="const", bufs=1))
        big = ctx.enter_context(tc.tile_pool(name="big", bufs=1))
        xno = ctx.enter_context(tc.tile_pool(name="xno", bufs=2))
        epool = ctx.enter_context(tc.tile_pool(name="epool", bufs=18))
        small = ctx.enter_context(tc.tile_pool(name="small", bufs=1))
        mmps = ctx.enter_context(
            tc.tile_pool(name="mmps", bufs=2, space=bass.MemorySpace.PSUM)
        )
        pvps = ctx.enter_context(
            tc.tile_pool(name="pvps", bufs=2, space=bass.MemorySpace.PSUM)
        )

        # ---- load inputs ----
        x_sb = []
        vecs_sb = []
        wqkvT_sb = []
        wprojT_sb = []
        gsel_sb = []
        for ct in range(CT):
            xt = big.tile([128, HW], F32, tag=f"x{ct}")
            nc.sync.dma_start(xt[:], x_d[ct * 128 : (ct + 1) * 128, :])
            x_sb.append(xt)
            vt = const.tile([128, 5], F32, tag=f"vecs{ct}")
            nc.sync.dma_start(vt[:], vecs_d[ct])
            vecs_sb.append(vt)
            wq = const.tile([128, 3 * C], F32R, tag=f"wqkv{ct}")
            nc.sync.dma_start(wq[:], wqkvT_d[ct * 128 : (ct + 1) * 128, :])
            wqkvT_sb.append(wq)
            wp = const.tile([128, C], F32R, tag=f"wproj{ct}")
            nc.sync.dma_start(wp[:], wprojT_d[ct * 128 : (ct + 1) * 128, :])
            wprojT_sb.append(wp)
            gs = const.tile([128, C], F32, tag=f"gsel{ct}")
            nc.sync.dma_start(gs[:], gsel_d[ct * 128 : (ct + 1) * 128, :])
            gsel_sb.append(gs)
        bsel_sb = const.tile([NH, C], F32R, tag="bsel")
        nc.sync.dma_start(bsel_sb[:], bsel_d[:])
        vmask_sb = const.tile([128, 4 * 128], BF16, tag="vmask")
        nc.sync.dma_start(vmask_sb[:], vmask_d[:])

        # ---- GroupNorm statistics ----
        # per-channel mean/var via bn_stats (9 subgroups of 256), then average
        # groups of 8 channels with the gsel matmul on [mean, E[x^2]]
        stats2 = []
        for ct in range(CT):
            st = small.tile([128, 9, 6], F32, tag=f"bnst{ct}")
            xr = x_sb[ct].rearrange("p (n f) -> p n f", f=256)
            for sg in range(9):
                nc.vector.bn_stats(st[:, sg, :], xr[:, sg, :])
            mv = small.tile([128, 2], F32, tag=f"mv{ct}")
            nc.vector.bn_aggr(mv[:], st[:])
            s2 = small.tile([128, 2], F32, tag=f"s2{ct}")
            nc.vector.tensor_copy(s2[:, 0:1], mv[:, 0:1])
            nc.vector.tensor_tensor(s2[:, 1:2], mv[:, 0:1], mv[:, 0:1], op=OP.mult)
            nc.vector.tensor_add(s2[:, 1:2], s2[:, 1:2], mv[:, 1:2])
            stats2.append(s2)

        eps_sb = small.tile([128, 1], F32, tag="eps")
        nc.vector.memset(eps_sb[:], EPS)
        A_sb = []
        B_sb = []
        for mb in range(CT):
            ps = mmps.tile([128, 2], F32, tag="mmps")
            for kt in range(CT):
                nc.tensor.matmul(
                    ps[:],
                    gsel_sb[kt][:, mb * 128 : (mb + 1) * 128],
                    stats2[kt][:],
                    start=(kt == 0),
                    stop=(kt == CT - 1),
                )
            rstd = small.tile([128, 1], F32, tag=f"rstd{mb}")
            msq = small.tile([128, 1], F32, tag=f"msq{mb}")
            mg = small.tile([128, 1], F32, tag=f"mg{mb}")
            nc.vector.tensor_copy(mg[:], ps[:, 0:1])
            nc.vector.tensor_tensor(msq[:], mg[:], mg[:], op=OP.mult)
            nc.vector.tensor_tensor(rstd[:], ps[:, 1:2], msq[:], op=OP.subtract)
            # rstd = 1/sqrt(var + eps)
            nc.scalar.activation(rstd[:], rstd[:], AF.Sqrt, bias=eps_sb[:])
            nc.vector.reciprocal(rstd[:], rstd[:])
            a = small.tile([128, 1], F32, tag=f"A{mb}")
            b = small.tile([128, 1], F32, tag=f"B{mb}")
            nc.vector.tensor_tensor(a[:], vecs_sb[mb][:, 0:1], rstd[:], op=OP.mult)
            nc.vector.tensor_tensor(b[:], mg[:], a[:], op=OP.mult)
            nc.vector.tensor_tensor(b[:], vecs_sb[mb][:, 1:2], b[:], op=OP.subtract)
            A_sb.append(a)
            B_sb.append(b)

        xn_sb = []
        for ct in range(CT):
            xn = xno.tile([128, HW], F32R, tag="xno")
            nc.vector.tensor_scalar(
                xn[:], x_sb[ct][:], A_sb[ct][:], B_sb[ct][:], op0=OP.mult, op1=OP.add
            )
            xn_sb.append(xn)

        # ---- qkv: q,k in channel-layout [256, HW]; v transposed [HW, 256] ----
        q_sb = [big.tile([128, HW], F32R, tag=f"q{ct}", name=f"q{ct}") for ct in range(CT)]
        k_sb = [big.tile([128, HW], F32R, tag=f"k{ct}", name=f"k{ct}") for ct in range(CT)]
        for which in range(2):  # 0 -> q, 1 -> k
            woff = which * C
            dst = q_sb if which == 0 else k_sb
            for mb in range(CT):
                for po, pl in PCH:
                    ps = mmps.tile([128, 1024], F32, tag="mmps")
                    for kt in range(CT):
                        nc.tensor.matmul(
                            ps[:, :pl],
                            wqkvT_sb[kt][
                                :, woff + mb * 128 : woff + (mb + 1) * 128
                            ],
                            xn_sb[kt][:, po : po + pl],
                            start=(kt == 0),
                            stop=(kt == CT - 1),
                        )
                    if which == 0:
                        # q = (q_raw + qb) / 8  (qb/8 is precomputed on host)
                        nc.vector.tensor_scalar(
                            dst[mb][:, po : po + pl],
                            ps[:, :pl],
                            0.125,
                            vecs_sb[mb][:, 2:3],
                            op0=OP.mult,
                            op1=OP.add,
                        )
                    else:
                        nc.vector.tensor_scalar(
                            dst[mb][:, po : po + pl],
                            ps[:, :pl],
                            vecs_sb[mb][:, 3:4],
                            None,
                            op0=OP.add,
                        )

        # vT augmented: per j-block a [128, 512] bf16 tile; head h occupies
        # cols h*128..h*128+127 = its 64 v-dims + a ones column + zeros
        vtaug = []
        for pb in range(PB):
            vt = big.tile([128, 4 * 128], BF16, tag=f"vt{pb}")
            nc.sync.dma_start(vt[:], vmask_sb[:])
            ps = mmps.tile([128, 1024], F32, tag="mmps")
            for kt in range(CT):
                nc.tensor.matmul(
                    ps[:, : C],
                    xn_sb[kt][:, pb * 128 : (pb + 1) * 128],
                    wqkvT_sb[kt][:, 2 * C : 3 * C],
                    start=(kt == 0),
                    stop=(kt == CT - 1),
                )
            for h in range(NH):
                dcol = h * 128 + (0 if h % 2 == 0 else 64)
                nc.vector.tensor_copy(
                    vt[:, dcol : dcol + 64], ps[:, h * 64 : (h + 1) * 64]
                )
            vtaug.append(vt)

        # ---- attention ----
        attn_sb = [big.tile([128, HW], F32R, tag=f"attn{p}", name=f"attn{p}") for p in range(CT)]
        stage = small.tile([128, HW], F32, tag="stage")
        for h in range(NH):
            ct = h // 2
            ro = (h % 2) * 64
            srow = ONES_COL[h]
            for io, il in ICH:
                es = []
                for pb in range(PB):
                    st = mmps.tile([128, 1024], F32, tag="mmps")
                    for co, cl in _chunks(il):
                        nc.tensor.matmul(
                            st[:, co : co + cl],
                            k_sb[ct][ro : ro + 64, pb * 128 : (pb + 1) * 128],
                            q_sb[ct][ro : ro + 64, io + co : io + co + cl],
                            start=True,
                            stop=True,
                        )
                    e = epool.tile([128, il], BF16, tag="E")
                    nc.scalar.activation(e[:], st[:, :il], AF.Exp)
                    es.append(e)
                pv = pvps.tile([128, 1024], F32, tag="pvps")
                for pb in range(PB):
                    for co, cl in _chunks(il):
                        nc.tensor.matmul(
                            pv[:, co : co + cl],
                            vtaug[pb][:, h * 128 : (h + 1) * 128],
                            es[pb][:, co : co + cl],
                            start=(pb == 0),
                            stop=(pb == PB - 1),
                        )
                nc.vector.tensor_copy(
                    attn_sb[ct][ro : ro + 64, io : io + il], pv[ro : ro + 64, :il]
                )
                nc.vector.tensor_copy(
                    stage[srow : srow + 1, io : io + il], pv[srow : srow + 1, :il]
                )

        # ---- softmax denominators -> per-pixel reciprocal, broadcast ----
        with nc.allow_low_precision(reason="f32r rounding of softmax denominators"):
            for h in range(NH):
                r = ONES_COL[h]
                nc.vector.reciprocal(
                    stage[r : r + 1, :].bitcast(F32R), stage[r : r + 1, :]
                )
        rsum = small.tile([NH, HW], F32R, tag="rsum")
        for h in range(NH):
            nc.sync.dma_start(
                rsum[h : h + 1, :],
                stage[ONES_COL[h] : ONES_COL[h] + 1, :].bitcast(F32R),
            )
        for p in range(CT):
            for po, pl in PCH:
                rs = mmps.tile([128, 1024], F32, tag="mmps")
                nc.tensor.matmul(
                    rs[:, :pl],
                    bsel_sb[:, p * 128 : (p + 1) * 128],
                    rsum[:, po : po + pl],
                    start=True,
                    stop=True,
                )
                nc.vector.tensor_tensor(
                    attn_sb[p][:, po : po + pl],
                    attn_sb[p][:, po : po + pl],
                    rs[:, :pl],
                    op=OP.mult,
                )

        # ---- proj + residual ----
        for ct in range(CT):
            ot = xno.tile([128, HW], F32, tag="xno")
            nc.vector.tensor_scalar(
                ot[:], x_sb[ct][:], vecs_sb[ct][:, 4:5], None, op0=OP.add
            )
            for po, pl in PCH:
                ps = mmps.tile([128, 1024], F32, tag="mmps")
                for kt in range(CT):
                    nc.tensor.matmul(
                        ps[:, :pl],
                        wprojT_sb[kt][:, ct * 128 : (ct + 1) * 128],
                        attn_sb[kt][:, po : po + pl],
                        start=(kt == 0),
                        stop=(kt == CT - 1),
                    )
                nc.vector.tensor_tensor(
                    ot[:, po : po + pl], ot[:, po : po + pl], ps[:, :pl], op=OP.add
                )
            nc.sync.dma_start(out_d[ct * 128 : (ct + 1) * 128, :], ot[:])

    nc.compile()
    return nc


_NC = None


def _get_nc():
    global _NC
    if _NC is None:
        _NC = _build()
    return _NC


def _host_prep(x, gn_w, gn_b, qkv_w, qkv_b, proj_w, proj_b):
    f32 = np.float32
    x = np.asarray(x, dtype=f32)
    gn_w = np.asarray(gn_w, dtype=f32)
    gn_b = np.asarray(gn_b, dtype=f32)
    qkv_w = np.asarray(qkv_w, dtype=f32)
    qkv_b = np.asarray(qkv_b, dtype=f32)
    proj_w = np.asarray(proj_w, dtype=f32)
    proj_b = np.asarray(proj_b, dtype=f32)

    b = x.shape[0]
    xs = np.ascontiguousarray(x.reshape(b, C, HW))

    wqkvT = np.ascontiguousarray(qkv_w.T)
    wprojT = np.ascontiguousarray(proj_w.T)

    gsel = np.zeros((C, C), dtype=f32)
    for g in range(G):
        gsel[g * 8 : (g + 1) * 8, g * 8 : (g + 1) * 8] = 1.0 / 8.0

    bsel = np.zeros((NH, C), dtype=f32)
    for h in range(NH):
        bsel[h, h * HD : (h + 1) * HD] = 1.0

    pbeff = proj_b + proj_w @ qkv_b[2 * C : 3 * C]
    vecs = np.stack(
        [gn_w, gn_b, qkv_b[:C] / 8.0, qkv_b[C : 2 * C], pbeff], axis=-1
    ).reshape(CT, 128, 5)
    vecs = np.ascontiguousarray(vecs.astype(f32))

    vmask = np.zeros((128, 4 * 128), dtype=np.float32)
    for h in range(NH):
        vmask[:, h * 128 + ONES_COL[h]] = 1.0
    import ml_dtypes

    vmask = vmask.astype(ml_dtypes.bfloat16)

    shared = {
        "wqkvT": wqkvT,
        "wprojT": wprojT,
        "gsel": gsel,
        "bsel": bsel,
        "vecs": vecs,
        "vmask": vmask,
    }
    in_maps = [dict(shared, x=np.ascontiguousarray(xs[i])) for i in range(b)]
    return in_maps, x.shape


def _run(inputs, **run_kwargs):
    nc = _get_nc()
    in_maps, xshape = _host_prep(**inputs)
    res = run_bass_kernel_spmd(
        nc, in_maps, core_ids=list(range(N_CORES)), **run_kwargs
    )
    out = np.stack([res.results[i]["out"] for i in range(N_CORES)])
    return out.reshape(xshape).astype(np.float32), res


def kernel(**inputs):
    out, _ = _run(inputs)
    return out


if __name__ == "__main__":
    rng = np.random.default_rng(0)
    ins = {
        "x": rng.standard_normal((8, C, 48, 48), dtype=np.float32),
        "gn_w": rng.random(C, dtype=np.float32),
        "gn_b": rng.standard_normal(C, dtype=np.float32) * 0.02,
        "qkv_w": (rng.standard_normal((3 * C, C), dtype=np.float32) / 16.0),
        "qkv_b": rng.standard_normal(3 * C, dtype=np.float32) * 0.02,
        "proj_w": (rng.standard_normal((C, C), dtype=np.float32) / 16.0),
        "proj_b": rng.standard_normal(C, dtype=np.float32) * 0.02,
    }
    out = kernel(**ins)
    print("out", out.shape, out.dtype, float(np.abs(out).mean()))
